# revision 1
# baseline (speedup 1.0000x reference)
import sys
sys.path.insert(0, "/opt/trn_rl_repo")
import numpy as np

B, T, C, H = 2, 2048, 1024, 16
D = C // H          # 64
HPC = 4             # heads per core
OC = HPC * D        # 256 out channels per core
NEG = -1e30

_cache = {}


def _build_nc():
    import concourse.mybir as mybir
    from concourse import bacc
    from concourse.tile import TileContext

    f32 = mybir.dt.float32
    f32r = mybir.dt.float32r
    Exp = mybir.ActivationFunctionType.Exp

    nc = bacc.Bacc("TRN2", target_bir_lowering=False)

    xt = nc.dram_tensor("xt", [C, T], f32r, kind="ExternalInput")
    wqe = nc.dram_tensor("wqe", [C, 128], f32r, kind="ExternalInput")
    wqo = nc.dram_tensor("wqo", [C, 128], f32r, kind="ExternalInput")
    wke = nc.dram_tensor("wke", [C, 128], f32r, kind="ExternalInput")
    wko = nc.dram_tensor("wko", [C, 128], f32r, kind="ExternalInput")
    wv = nc.dram_tensor("wv", [C, OC], f32r, kind="ExternalInput")
    wos = nc.dram_tensor("wos", [OC, C], f32r, kind="ExternalInput")
    cosd = nc.dram_tensor("cos", [128, T], f32r, kind="ExternalInput")
    sind = nc.dram_tensor("sin", [128, T], f32r, kind="ExternalInput")
    identd = nc.dram_tensor("ident", [128, 128], f32r, kind="ExternalInput")
    trid = nc.dram_tensor("tri", [128, 128], f32r, kind="ExternalInput")
    onesd = nc.dram_tensor("ones", [128, 64], f32r, kind="ExternalInput")
    y = nc.dram_tensor("y", [T, C], f32, kind="ExternalOutput")

    NT = T // 512    # 4 big t-blocks
    NTB = T // 128   # 16 j-chunks / t128 blocks
    NCH = C // 128   # 8 contraction chunks

    with TileContext(nc) as tc:
        with tc.tile_pool(name="wgt", bufs=1) as wgt, \
             tc.tile_pool(name="persist", bufs=1) as persist:
            w_qe = wgt.tile([128, NCH, 128], f32r, tag="wqe")
            w_qo = wgt.tile([128, NCH, 128], f32r, tag="wqo")
            w_ke = wgt.tile([128, NCH, 128], f32r, tag="wke")
            w_ko = wgt.tile([128, NCH, 128], f32r, tag="wko")
            w_v = wgt.tile([128, NCH, OC], f32r, tag="wv")
            w_o = wgt.tile([128, 2, C], f32r, tag="wo")
            for ci in range(NCH):
                nc.scalar.dma_start(out=w_qe[:, ci], in_=wqe.rearrange("(a p) m -> a p m", p=128)[ci])
                nc.sync.dma_start(out=w_qo[:, ci], in_=wqo.rearrange("(a p) m -> a p m", p=128)[ci])
                nc.gpsimd.dma_start(out=w_ke[:, ci], in_=wke.rearrange("(a p) m -> a p m", p=128)[ci])
                nc.sync.dma_start(out=w_ko[:, ci], in_=wko.rearrange("(a p) m -> a p m", p=128)[ci])
                nc.sync.dma_start(out=w_v[:, ci], in_=wv.rearrange("(a p) m -> a p m", p=128)[ci])
            for k in range(2):
                nc.sync.dma_start(out=w_o[:, k], in_=wos.rearrange("(a p) m -> a p m", p=128)[k])
            t_cos = wgt.tile([128, T], f32r, tag="cos")
            t_sin = wgt.tile([128, T], f32r, tag="sin")
            t_id = wgt.tile([128, 128], f32r, tag="id")
            t_tri = wgt.tile([128, 128], f32r, tag="tri")
            t_one = wgt.tile([128, 64], f32r, tag="one")
            nc.sync.dma_start(out=t_one, in_=onesd[:, :])
            nc.sync.dma_start(out=t_cos, in_=cosd[:, :])
            nc.sync.dma_start(out=t_sin, in_=sind[:, :])
            nc.sync.dma_start(out=t_id, in_=identd[:, :])
            nc.sync.dma_start(out=t_tri, in_=trid[:, :])

            rQ = persist.tile([128, 2, T], f32r, tag="rq")
            rK = persist.tile([128, 2, T], f32r, tag="rk")
            Vt = persist.tile([128, NTB, HPC, D + 1], f32r, tag="v")
            nc.sync.dma_start(out=Vt[:, :, :, D:D + 1],
                              in_=onesd.rearrange("p (a b c) -> p a b c", a=NTB, b=HPC)[:, :, :, :])

            xtr = xt.rearrange("(a p) t -> a p t", p=128)

            # ---------------- projections + rope (pair layout) --------------
            with tc.tile_pool(name="pair", bufs=1) as pairp:
                pQe = pairp.tile([128, T], f32r, tag="pqe")
                pQo = pairp.tile([128, T], f32r, tag="pqo")
                pKe = pairp.tile([128, T], f32r, tag="pke")
                pKo = pairp.tile([128, T], f32r, tag="pko")
                pair_of = {"qe": pQe, "qo": pQo, "ke": pKe, "ko": pKo}
                qk_scope = tc.tile_pool(name="xtp", bufs=13)
                xtp = qk_scope.__enter__()
                qk_ps_cm = tc.tile_pool(name="qk_ps", bufs=1, space="PSUM")
                qk_ps = qk_ps_cm.__enter__()
                v_ps_cm = tc.tile_pool(name="v_ps", bufs=2, space="PSUM")
                v_ps = v_ps_cm.__enter__()
                rope_cm = tc.tile_pool(name="rope_t", bufs=2)
                rope_t = rope_cm.__enter__()
                for tb in range(NT):
                    ts = slice(tb * 512, tb * 512 + 512)
                    xts = []
                    qs = [nc.sync, nc.scalar, nc.gpsimd, nc.sync]
                    for ci in range(NCH):
                        xtile = xtp.tile([128, 512], f32r, tag="xt")
                        qs[ci % 4].dma_start(out=xtile, in_=xtr[ci, :, ts])
                        xts.append(xtile)
                    ps = {}
                    for nm, w in (("qe", w_qe), ("qo", w_qo),
                                  ("ke", w_ke), ("ko", w_ko)):
                        p = qk_ps.tile([128, 512], f32, tag="ps" + nm)
                        for ci in range(NCH):
                            nc.tensor.matmul(p, w[:, ci], xts[ci],
                                             start=(ci == 0), stop=(ci == NCH - 1))
                        ps[nm] = p
                    # rope: muls on vector (PSUM reads), add/sub on gpsimd
                    for nm in ("q", "k"):
                        e, o = ps[nm + "e"], ps[nm + "o"]
                        pe, po = pair_of[nm + "e"], pair_of[nm + "o"]
                        t1 = rope_t.tile([128, 512], f32, tag="t1" + nm)
                        t2 = rope_t.tile([128, 512], f32, tag="t2" + nm)
                        nc.vector.tensor_mul(t1, e, t_cos[:, ts])
                        nc.vector.tensor_mul(t2, o, t_sin[:, ts])
                        nc.gpsimd.tensor_sub(pe[:, ts], t1, t2)
                        t3 = rope_t.tile([128, 512], f32, tag="t3" + nm)
                        t4 = rope_t.tile([128, 512], f32, tag="t4" + nm)
                        nc.vector.tensor_mul(t3, o, t_cos[:, ts])
                        nc.vector.tensor_mul(t4, e, t_sin[:, ts])
                        nc.gpsimd.tensor_add(po[:, ts], t3, t4)
                    # V projection fused: reuse the same xt tiles
                    for s in range(4):
                        pv = v_ps.tile([128, OC], f32, tag="psv",
                                       name=f"pv_{tb}_{s}")
                        for ci in range(NCH):
                            nc.tensor.matmul(
                                pv, xts[ci][:, s * 128:s * 128 + 128], w_v[:, ci],
                                start=(ci == 0), stop=(ci == NCH - 1))
                        for h in range(HPC):
                            nc.vector.tensor_copy(
                                out=Vt[:, tb * 4 + s, h, 0:D],
                                in_=pv[:, h * D:h * D + D])
                rope_cm.__exit__(None, None, None)
                v_ps_cm.__exit__(None, None, None)
                qk_ps_cm.__exit__(None, None, None)
                qk_scope.__exit__(None, None, None)
                # repack pair -> head layout via DMA
                for a in range(HPC):
                    ob, rb = a // 2, 64 * (a % 2)
                    nc.scalar.dma_start(out=rQ[rb:rb + 32, ob, :], in_=pQe[32 * a:32 * a + 32, :])
                    nc.gpsimd.dma_start(out=rQ[rb + 32:rb + 64, ob, :], in_=pQo[32 * a:32 * a + 32, :])
                    nc.sync.dma_start(out=rK[rb:rb + 32, ob, :], in_=pKe[32 * a:32 * a + 32, :])
                    nc.sync.dma_start(out=rK[rb + 32:rb + 64, ob, :], in_=pKo[32 * a:32 * a + 32, :])

            # ---------------- attention, per head ----------------
            OCt = persist.tile([128, 2, T], f32r, tag="oc")
            with tc.tile_pool(name="st_ps", bufs=2, space="PSUM") as st_ps, \
                 tc.tile_pool(name="ot_ps", bufs=4, space="PSUM") as ot_ps, \
                 tc.tile_pool(name="est", bufs=6) as estp, \
                 tc.tile_pool(name="nrm", bufs=6) as nrm:
                for h in range(HPC):
                    ob, rb = h // 2, 64 * (h % 2)
                    lQ = rQ[rb:rb + 64, ob, :]
                    lK = rK[rb:rb + 64, ob, :]
                    ot = [ot_ps.tile([D + 1, 512], f32, tag="ot", name=f"ot_h{h}_w{w}") for w in range(NT)]
                    est_of = {}
                    pend = []

                    def _emit_pv(c, h=h, ot=ot, est_of=est_of):
                        hs = 128 * c
                        nseg = 1 if T - hs <= 1024 else 2
                        for sg in range(nseg):
                            est, slo, w_seg = est_of[(c, sg)]
                            shi = slo + w_seg
                            for w in range(NT):
                                glo, ghi = 512 * w, 512 * w + 512
                                lo, hi = max(slo, glo), min(shi, ghi)
                                if lo >= hi:
                                    continue
                                nc.tensor.matmul(
                                    ot[w][:, lo - glo:hi - glo],
                                    Vt[:, c, h, :],
                                    est[:, lo - slo:hi - slo],
                                    start=(c == 0 and lo == glo),
                                    stop=(c == min(NTB - 1, 4 * w + 3) and hi == ghi),
                                    skip_group_check=True)
                    for c in range(NTB):
                        hs = 128 * c
                        iext = T - hs
                        nseg = 1 if iext <= 1024 else 2
                        for sg in range(nseg):
                            slo = hs + 1024 * sg              # global start
                            w_seg = min(1024, T - slo)
                            st = st_ps.tile([128, 1024], f32, tag="st")
                            # score matmuls in <=512 windows
                            off = 0
                            while off < w_seg:
                                n = min(512, w_seg - off)
                                nc.tensor.matmul(
                                    st[:, off:off + n],
                                    lK[:, hs:hs + 128],
                                    lQ[:, slo + off:slo + off + n],
                                    start=True, stop=(not (sg == 0 and off == 0)))
                                off += n
                            if sg == 0:
                                # causal mask on diagonal block
                                nc.tensor.matmul(
                                    st[:, 0:128], t_id, t_tri,
                                    start=False, stop=True, skip_group_check=True)
                            est = estp.tile([128, 1024], f32r, tag="est")
                            nc.scalar.activation(out=est[:, 0:w_seg],
                                                 in_=st[:, 0:w_seg],
                                                 func=Exp, scale=0.125)
                            est_of[(c, sg)] = (est, slo, w_seg)
                        # PV for the PREVIOUS chunk (software pipeline: keeps
                        # PE busy while ACT runs this chunk's exp)
                        pend.append(c)
                        if len(pend) > 1:
                            _emit_pv(pend.pop(0))
                    _emit_pv(pend.pop(0))
                    # normalize each window and place into OCt
                    for w in range(NT):
                        rl = nrm.tile([D + 1, 512], f32r, tag="rl")
                        with nc.allow_low_precision(reason="1/l in f32r feeds matmul"):
                            nc.vector.reciprocal(out=rl[D:D + 1, :], in_=ot[w][D:D + 1, :])
                        rlb = st_ps.tile([D, 512], f32, tag="st", name=f"rlb_h{h}_w{w}")
                        nc.tensor.matmul(rlb, t_one[D:D + 1, :], rl[D:D + 1, :],
                                         start=True, stop=True)
                        rlb_sb = nrm.tile([D, 512], f32, tag="rlbsb")
                        nc.vector.tensor_copy(out=rlb_sb, in_=rlb)
                        otn = nrm.tile([D, 512], f32r, tag="otn")
                        nc.vector.tensor_mul(otn, ot[w][0:D, :], rlb_sb)
                        nc.sync.dma_start(
                            out=OCt[rb:rb + D, ob, 512 * w:512 * w + 512], in_=otn)

            # ---------------- output projection ----------------
            with tc.tile_pool(name="y_ps", bufs=4, space="PSUM") as y_ps, \
                 tc.tile_pool(name="y_sb", bufs=4) as y_sb:
                for tb in range(NTB):
                    tsl = slice(tb * 128, tb * 128 + 128)
                    for ob2 in range(2):
                        py = y_ps.tile([128, 512], f32, tag="py")
                        for k in range(2):
                            nc.tensor.matmul(
                                py, OCt[:, k, tsl],
                                w_o[:, k, 512 * ob2:512 * ob2 + 512],
                                start=(k == 0), stop=(k == 1))
                        sy = y_sb.tile([128, 512], f32, tag="sy")
                        nc.vector.tensor_copy(out=sy, in_=py)
                        (nc.sync if ob2 == 0 else nc.scalar).dma_start(
                            out=y[tsl, 512 * ob2:512 * ob2 + 512], in_=sy)
    nc.compile()
    return nc


def _prep_inputs(x, wq, wk, wv, wo, rope_cos, rope_sin):
    """Host-side sharding/pre-transposition. Core i: batch i//4, head group i%4."""
    f = np.float32
    cosT = np.ascontiguousarray(rope_cos.T.astype(f))       # [32, T]
    sinT = np.ascontiguousarray(rope_sin.T.astype(f))
    COS = np.tile(cosT, (4, 1))                              # [128, T]
    SIN = np.tile(sinT, (4, 1))
    ident = np.eye(128, dtype=f)
    tri = np.where(np.arange(128)[:, None] > np.arange(128)[None, :],
                   f(NEG), f(0.0))
    xT = [np.ascontiguousarray(x[b].T.astype(f)) for b in range(B)]
    in_maps = []
    for core in range(8):
        b, g = core // 4, core % 4
        heads = [4 * g + a for a in range(HPC)]
        e_rows = np.concatenate([64 * h + 2 * np.arange(32) for h in heads])
        o_rows = e_rows + 1
        sl = slice(OC * g, OC * g + OC)
        in_maps.append({
            "xt": xT[b],
            "wqe": np.ascontiguousarray(wq[e_rows].T.astype(f)),
            "wqo": np.ascontiguousarray(wq[o_rows].T.astype(f)),
            "wke": np.ascontiguousarray(wk[e_rows].T.astype(f)),
            "wko": np.ascontiguousarray(wk[o_rows].T.astype(f)),
            "wv": np.ascontiguousarray(wv[sl].T.astype(f)),
            "wos": np.ascontiguousarray(wo[:, sl].T.astype(f)),
            "cos": COS, "sin": SIN, "ident": ident, "tri": tri,
            "ones": np.ones((128, 64), np.float32),
        })
    return in_maps


def kernel(x, wq, wk, wv, wo, rope_cos, rope_sin, _trace=False):
    from concourse.bass_utils import run_bass_kernel_spmd
    if "nc" not in _cache:
        _cache["nc"] = _build_nc()
    nc = _cache["nc"]
    in_maps = _prep_inputs(np.asarray(x), np.asarray(wq), np.asarray(wk),
                           np.asarray(wv), np.asarray(wo),
                           np.asarray(rope_cos), np.asarray(rope_sin))
    res = run_bass_kernel_spmd(nc, in_maps, core_ids=list(range(8)),
                               trace=_trace)
    _cache["last_result"] = res
    out = np.zeros((B, T, C), np.float32)
    for core in range(8):
        out[core // 4] += res.results[core]["y"]
    return out



# revision 7
# speedup vs baseline: 1.1727x; 1.1727x over previous
import sys
sys.path.insert(0, "/opt/trn_rl_repo")
import numpy as np

B, T, C, H = 2, 2048, 1024, 16
D = C // H          # 64
HPC = 4             # heads per core
OC = HPC * D        # 256 out channels per core
NEG = -1e30

_cache = {}


def _build_nc():
    import concourse.mybir as mybir
    from concourse import bacc
    from concourse.tile import TileContext

    f32 = mybir.dt.float32
    f32r = mybir.dt.float32r
    bf16 = mybir.dt.bfloat16
    Exp = mybir.ActivationFunctionType.Exp

    nc = bacc.Bacc("TRN2", target_bir_lowering=False)

    xt = nc.dram_tensor("xt", [C, T], bf16, kind="ExternalInput")
    wqe = nc.dram_tensor("wqe", [C, 128], bf16, kind="ExternalInput")
    wqo = nc.dram_tensor("wqo", [C, 128], bf16, kind="ExternalInput")
    wke = nc.dram_tensor("wke", [C, 128], bf16, kind="ExternalInput")
    wko = nc.dram_tensor("wko", [C, 128], bf16, kind="ExternalInput")
    wv = nc.dram_tensor("wv", [C, OC], bf16, kind="ExternalInput")
    wos = nc.dram_tensor("wos", [OC, C], bf16, kind="ExternalInput")
    cosd = nc.dram_tensor("cos", [128, T], f32r, kind="ExternalInput")
    sind = nc.dram_tensor("sin", [128, T], f32r, kind="ExternalInput")
    identd = nc.dram_tensor("ident", [128, 128], bf16, kind="ExternalInput")
    trid = nc.dram_tensor("tri", [128, 128], bf16, kind="ExternalInput")
    onesd = nc.dram_tensor("ones", [128, 64], f32r, kind="ExternalInput")
    pmed = nc.dram_tensor("pme", [128, 256], f32r, kind="ExternalInput")
    pmod = nc.dram_tensor("pmo", [128, 256], f32r, kind="ExternalInput")
    y = nc.dram_tensor("y", [T, C], bf16, kind="ExternalOutput")

    NT = T // 512    # 4 big t-blocks
    NTB = T // 128   # 16 j-chunks / t128 blocks
    NCH = C // 128   # 8 contraction chunks

    with TileContext(nc) as tc:
        with tc.tile_pool(name="wgt", bufs=1) as wgt, \
             tc.tile_pool(name="persist", bufs=1) as persist:
            Xt = wgt.tile([128, NCH, NT, 512], bf16, tag="x")
            w_qe = wgt.tile([128, NCH, 128], bf16, tag="wqe")
            w_qo = wgt.tile([128, NCH, 128], bf16, tag="wqo")
            w_ke = wgt.tile([128, NCH, 128], bf16, tag="wke")
            w_ko = wgt.tile([128, NCH, 128], bf16, tag="wko")
            w_v = wgt.tile([128, NCH, OC], bf16, tag="wv")
            w_o = wgt.tile([128, 2, C], bf16, tag="wo")
            t_cos = wgt.tile([128, T], f32r, tag="cos")
            t_sin = wgt.tile([128, T], f32r, tag="sin")
            t_id = wgt.tile([128, 128], bf16, tag="id")
            t_tri = wgt.tile([128, 128], bf16, tag="tri")
            t_one = wgt.tile([128, 64], f32r, tag="one")
            t_me = wgt.tile([128, 256], f32r, tag="me")
            t_mo = wgt.tile([128, 256], f32r, tag="mo")

            # DMA issue order matters: x(tb0) + qk weights first so PE can
            # start, everything else behind them.
            xr = xt.rearrange("(a p) t -> p a t", p=128)
            nc.sync.dma_start(out=Xt[:, :, 0], in_=xr[:, :, 0:512])
            nc.scalar.dma_start(out=w_qe, in_=wqe.rearrange("(a p) m -> p a m", p=128))
            nc.scalar.dma_start(out=w_qo, in_=wqo.rearrange("(a p) m -> p a m", p=128))
            nc.scalar.dma_start(out=w_ke, in_=wke.rearrange("(a p) m -> p a m", p=128))
            nc.scalar.dma_start(out=w_ko, in_=wko.rearrange("(a p) m -> p a m", p=128))
            nc.scalar.dma_start(out=w_v, in_=wv.rearrange("(a p) m -> p a m", p=128))
            nc.scalar.dma_start(out=t_cos, in_=cosd[:, :])
            nc.scalar.dma_start(out=t_sin, in_=sind[:, :])
            nc.scalar.dma_start(out=t_me, in_=pmed[:, :])
            nc.scalar.dma_start(out=t_mo, in_=pmod[:, :])
            nc.scalar.dma_start(out=t_id, in_=identd[:, :])
            nc.scalar.dma_start(out=t_tri, in_=trid[:, :])
            nc.scalar.dma_start(out=t_one, in_=onesd[:, :])
            for tb in range(1, NT):
                nc.sync.dma_start(out=Xt[:, :, tb],
                                  in_=xr[:, :, tb * 512:tb * 512 + 512])
            nc.scalar.dma_start(out=w_o, in_=wos.rearrange("(a p) m -> p a m", p=128))

            rQ = persist.tile([128, 2, T], f32r, tag="rq")
            rK = persist.tile([128, 2, T], f32r, tag="rk")
            Vt = persist.tile([128, NTB, HPC, D + 1], bf16, tag="v")
            OCt = persist.tile([128, 2, T], bf16, tag="oc")
            nc.vector.memset(Vt[:, :, :, D:D + 1], 1.0)

            # ---------------- projections + rope + repack --------------
            with tc.tile_pool(name="qk_ps", bufs=1, space="PSUM") as qk_ps, \
                 tc.tile_pool(name="v_ps", bufs=2, space="PSUM") as v_ps, \
                 tc.tile_pool(name="rp_ps", bufs=2, space="PSUM") as rp_ps, \
                 tc.tile_pool(name="pair", bufs=2) as pairp, \
                 tc.tile_pool(name="rope_t", bufs=2) as rope_t:
                for tb in range(NT):
                    ts = slice(tb * 512, tb * 512 + 512)
                    ps = {}
                    for nm, w in (("qe", w_qe), ("qo", w_qo),
                                  ("ke", w_ke), ("ko", w_ko)):
                        p = qk_ps.tile([128, 512], f32, tag="ps" + nm)
                        for ci in range(NCH):
                            nc.tensor.matmul(p, w[:, ci], Xt[:, ci, tb],
                                             start=(ci == 0), stop=(ci == NCH - 1))
                        ps[nm] = p
                    # rope: muls on vector (PSUM reads), add/sub on gpsimd
                    pair = {}
                    for nm in ("q", "k"):
                        e, o = ps[nm + "e"], ps[nm + "o"]
                        pe = pairp.tile([128, 512], f32r, tag="p" + nm + "e")
                        po = pairp.tile([128, 512], f32r, tag="p" + nm + "o")
                        t1 = rope_t.tile([128, 512], f32, tag="t1" + nm)
                        t2 = rope_t.tile([128, 512], f32, tag="t2" + nm)
                        nc.vector.tensor_mul(t1, e, t_cos[:, ts])
                        nc.vector.tensor_mul(t2, o, t_sin[:, ts])
                        nc.gpsimd.tensor_sub(pe, t1, t2)
                        t3 = rope_t.tile([128, 512], f32, tag="t3" + nm)
                        t4 = rope_t.tile([128, 512], f32, tag="t4" + nm)
                        nc.vector.tensor_mul(t3, o, t_cos[:, ts])
                        nc.vector.tensor_mul(t4, e, t_sin[:, ts])
                        nc.gpsimd.tensor_add(po, t3, t4)
                        pair[nm] = (pe, po)
                    # V projection: reuse the same Xt slices
                    for s in range(4):
                        pv = v_ps.tile([128, OC], f32, tag="psv",
                                       name=f"pv_{tb}_{s}")
                        for ci in range(NCH):
                            nc.tensor.matmul(
                                pv, Xt[:, ci, tb, s * 128:s * 128 + 128], w_v[:, ci],
                                start=(ci == 0), stop=(ci == NCH - 1))
                        for h in range(HPC):
                            nc.scalar.copy(
                                out=Vt[:, tb * 4 + s, h, 0:D],
                                in_=pv[:, h * D:h * D + D])
                    # repack pair -> head layout via PE permutation matmuls
                    for nm, dst in (("q", rQ), ("k", rK)):
                        pe_t, po_t = pair[nm]
                        for ob in range(2):
                            osl = slice(128 * ob, 128 * ob + 128)
                            pr = rp_ps.tile([128, 512], f32, tag="rp",
                                            name=f"rp_{nm}_{tb}_{ob}")
                            nc.tensor.matmul(pr, t_me[:, osl], pe_t,
                                             start=True, stop=False)
                            nc.tensor.matmul(pr, t_mo[:, osl], po_t,
                                             start=False, stop=True)
                            nc.scalar.copy(out=dst[:, ob, ts], in_=pr)

            # ---------------- attention, per head ----------------
            with tc.tile_pool(name="st_ps", bufs=2, space="PSUM") as st_ps, \
                 tc.tile_pool(name="ot_ps", bufs=4, space="PSUM") as ot_ps, \
                 tc.tile_pool(name="est", bufs=6) as estp, \
                 tc.tile_pool(name="nrm", bufs=3) as nrm:
                for h in range(HPC):
                    ob, rb = h // 2, 64 * (h % 2)
                    lQ = rQ[rb:rb + 64, ob, :]
                    lK = rK[rb:rb + 64, ob, :]
                    ot = [ot_ps.tile([D + 1, 512], f32, tag="ot", name=f"ot_h{h}_w{w}") for w in range(NT)]
                    est_of = {}
                    pend = []

                    def _emit_pv(c, h=h, ot=ot, est_of=est_of):
                        hs = 128 * c
                        nseg = 1 if T - hs <= 1024 else 2
                        for sg in range(nseg):
                            est, slo, w_seg = est_of[(c, sg)]
                            shi = slo + w_seg
                            for w in range(NT):
                                glo, ghi = 512 * w, 512 * w + 512
                                lo, hi = max(slo, glo), min(shi, ghi)
                                if lo >= hi:
                                    continue
                                nc.tensor.matmul(
                                    ot[w][:, lo - glo:hi - glo],
                                    Vt[:, c, h, :],
                                    est[:, lo - slo:hi - slo],
                                    start=(c == 0 and lo == glo),
                                    stop=(c == min(NTB - 1, 4 * w + 3) and hi == ghi),
                                    skip_group_check=True)

                    def _norm(w, h=h, ot=ot, ob=ob, rb=rb):
                        # normalize window w and place into OCt (bf16)
                        rl = nrm.tile([D + 1, 512], f32r, tag="rl")
                        with nc.allow_low_precision(reason="1/l in f32r feeds matmul"):
                            nc.vector.reciprocal(out=rl[D:D + 1, :], in_=ot[w][D:D + 1, :])
                        rlb = st_ps.tile([D, 512], f32, tag="st", name=f"rlb_h{h}_w{w}")
                        nc.tensor.matmul(rlb, t_one[D:D + 1, :], rl[D:D + 1, :],
                                         start=True, stop=True)
                        rlb_sb = nrm.tile([D, 512], f32, tag="rlbsb")
                        nc.vector.tensor_copy(out=rlb_sb, in_=rlb)
                        otn = nrm.tile([D, 512], bf16, tag="otn")
                        nc.vector.tensor_mul(otn, ot[w][0:D, :], rlb_sb)
                        nc.sync.dma_start(
                            out=OCt[rb:rb + D, ob, 512 * w:512 * w + 512],
                            in_=otn)

                    for c in range(NTB):
                        hs = 128 * c
                        iext = T - hs
                        nseg = 1 if iext <= 1024 else 2
                        for sg in range(nseg):
                            slo = hs + 1024 * sg              # global start
                            w_seg = min(1024, T - slo)
                            st = st_ps.tile([128, 1024], f32, tag="st")
                            # score matmuls in <=512 windows
                            off = 0
                            while off < w_seg:
                                n = min(512, w_seg - off)
                                nc.tensor.matmul(
                                    st[:, off:off + n],
                                    lK[:, hs:hs + 128],
                                    lQ[:, slo + off:slo + off + n],
                                    start=True, stop=(not (sg == 0 and off == 0)))
                                off += n
                            if sg == 0:
                                # causal mask on diagonal block (bf16: 1 cyc/row)
                                nc.tensor.matmul(
                                    st[:, 0:128], t_id, t_tri,
                                    start=False, stop=True, skip_group_check=True)
                            est = estp.tile([128, 1024], bf16, tag="est")
                            nc.scalar.activation(out=est[:, 0:w_seg],
                                                 in_=st[:, 0:w_seg],
                                                 func=Exp, scale=0.125)
                            est_of[(c, sg)] = (est, slo, w_seg)
                        # PV for the PREVIOUS chunk (software pipeline: keeps
                        # PE busy while ACT runs this chunk's exp); normalize
                        # each window as soon as its accumulation stops.
                        pend.append(c)
                        if len(pend) > 1:
                            cc = pend.pop(0)
                            _emit_pv(cc)
                            if cc >= 3 and (cc - 3) % 4 == 0:
                                _norm((cc - 3) // 4)
                    cc = pend.pop(0)
                    _emit_pv(cc)
                    _norm(3)

            # ---------------- output projection ----------------
            with tc.tile_pool(name="y_ps", bufs=4, space="PSUM") as y_ps, \
                 tc.tile_pool(name="y_sb", bufs=4) as y_sb:
                for tb in range(NTB):
                    tsl = slice(tb * 128, tb * 128 + 128)
                    for ob2 in range(2):
                        py = y_ps.tile([128, 512], f32, tag="py")
                        for k in range(2):
                            nc.tensor.matmul(
                                py, OCt[:, k, tsl],
                                w_o[:, k, 512 * ob2:512 * ob2 + 512],
                                start=(k == 0), stop=(k == 1))
                        sy = y_sb.tile([128, 512], bf16, tag="sy")
                        if ob2 == 0:
                            nc.vector.tensor_copy(out=sy, in_=py)
                        else:
                            nc.scalar.copy(out=sy, in_=py)
                        (nc.sync if ob2 == 0 else nc.scalar).dma_start(
                            out=y[tsl, 512 * ob2:512 * ob2 + 512], in_=sy)
    nc.compile()
    return nc


def _prep_inputs(x, wq, wk, wv, wo, rope_cos, rope_sin):
    """Host-side sharding/pre-transposition. Core i: batch i//4, head group i%4."""
    import ml_dtypes
    f = np.float32
    bf = ml_dtypes.bfloat16
    cosT = np.ascontiguousarray(rope_cos.T.astype(f))       # [32, T]
    sinT = np.ascontiguousarray(rope_sin.T.astype(f))
    COS = np.tile(cosT, (4, 1))                              # [128, T]
    SIN = np.tile(sinT, (4, 1))
    ident = np.eye(128, dtype=f).astype(bf)
    tri = np.where(np.arange(128)[:, None] > np.arange(128)[None, :],
                   f(NEG), f(0.0)).astype(bf)
    # repack permutation for plane ob (cols ob*128..): input partition
    # 64*ob+j with j<32 -> head 2ob slab, j>=32 -> head 2ob+1 slab.
    # evens land at [0:32] / [64:96], odds at [32:64] / [96:128].
    pme = np.zeros((128, 256), f)
    pmo = np.zeros((128, 256), f)
    r = np.arange(32)
    for ob in range(2):
        pme[64 * ob + r, 128 * ob + r] = 1.0
        pme[64 * ob + 32 + r, 128 * ob + 64 + r] = 1.0
        pmo[64 * ob + r, 128 * ob + 32 + r] = 1.0
        pmo[64 * ob + 32 + r, 128 * ob + 96 + r] = 1.0
    xT = [np.ascontiguousarray(x[b].T.astype(f)).astype(bf) for b in range(B)]
    in_maps = []
    for core in range(8):
        b, g = core // 4, core % 4
        heads = [4 * g + a for a in range(HPC)]
        e_rows = np.concatenate([64 * h + 2 * np.arange(32) for h in heads])
        o_rows = e_rows + 1
        sl = slice(OC * g, OC * g + OC)
        in_maps.append({
            "xt": xT[b],
            "wqe": np.ascontiguousarray(wq[e_rows].T.astype(f)).astype(bf),
            "wqo": np.ascontiguousarray(wq[o_rows].T.astype(f)).astype(bf),
            "wke": np.ascontiguousarray(wk[e_rows].T.astype(f)).astype(bf),
            "wko": np.ascontiguousarray(wk[o_rows].T.astype(f)).astype(bf),
            "wv": np.ascontiguousarray(wv[sl].T.astype(f)).astype(bf),
            "wos": np.ascontiguousarray(wo[:, sl].T.astype(f)).astype(bf),
            "cos": COS, "sin": SIN, "ident": ident, "tri": tri,
            "ones": np.ones((128, 64), np.float32),
            "pme": pme, "pmo": pmo,
        })
    return in_maps


def kernel(x, wq, wk, wv, wo, rope_cos, rope_sin, _trace=False):
    from concourse.bass_utils import run_bass_kernel_spmd
    if "nc" not in _cache:
        _cache["nc"] = _build_nc()
    nc = _cache["nc"]
    in_maps = _prep_inputs(np.asarray(x), np.asarray(wq), np.asarray(wk),
                           np.asarray(wv), np.asarray(wo),
                           np.asarray(rope_cos), np.asarray(rope_sin))
    res = run_bass_kernel_spmd(nc, in_maps, core_ids=list(range(8)),
                               trace=_trace)
    _cache["last_result"] = res
    out = np.zeros((B, T, C), np.float32)
    for core in range(8):
        out[core // 4] += np.asarray(res.results[core]["y"], np.float32)
    return out


# revision 15
# speedup vs baseline: 1.2385x; 1.0562x over previous
import sys
sys.path.insert(0, "/opt/trn_rl_repo")
import numpy as np

B, T, C, H = 2, 2048, 1024, 16
D = C // H          # 64
HPC = 4             # heads per core
OC = HPC * D        # 256 out channels per core
NEG = -1e30

_cache = {}


def _build_nc():
    import concourse.mybir as mybir
    from concourse import bacc
    from concourse.tile import TileContext

    f32 = mybir.dt.float32
    f32r = mybir.dt.float32r
    bf16 = mybir.dt.bfloat16
    Exp = mybir.ActivationFunctionType.Exp

    nc = bacc.Bacc("TRN2", target_bir_lowering=False)

    xt = nc.dram_tensor("xt", [C, T], bf16, kind="ExternalInput")
    wqe = nc.dram_tensor("wqe", [C, 128], bf16, kind="ExternalInput")
    wqo = nc.dram_tensor("wqo", [C, 128], bf16, kind="ExternalInput")
    wke = nc.dram_tensor("wke", [C, 128], bf16, kind="ExternalInput")
    wko = nc.dram_tensor("wko", [C, 128], bf16, kind="ExternalInput")
    wv = nc.dram_tensor("wv", [C, OC], bf16, kind="ExternalInput")
    wos = nc.dram_tensor("wos", [OC, C], bf16, kind="ExternalInput")
    cosd = nc.dram_tensor("cos", [32, T], f32r, kind="ExternalInput")
    sind = nc.dram_tensor("sin", [32, T], f32r, kind="ExternalInput")
    trid = nc.dram_tensor("tri01", [128, 128], bf16, kind="ExternalInput")
    onesd = nc.dram_tensor("ones", [128, 64], f32r, kind="ExternalInput")
    pmed = nc.dram_tensor("pme", [128, 256], f32r, kind="ExternalInput")
    pmod = nc.dram_tensor("pmo", [128, 256], f32r, kind="ExternalInput")
    y = nc.dram_tensor("y", [T, C], bf16, kind="ExternalOutput")

    NT = T // 512    # 4 big t-blocks
    NTB = T // 128   # 16 j-chunks / t128 blocks
    NCH = C // 128   # 8 contraction chunks

    with TileContext(nc) as tc:
        with tc.tile_pool(name="wgt", bufs=1) as wgt, \
             tc.tile_pool(name="persist", bufs=1) as persist:
            Xt = wgt.tile([128, NCH, NT, 512], bf16, tag="x")
            w_qe = wgt.tile([128, NCH, 128], bf16, tag="wqe")
            w_qo = wgt.tile([128, NCH, 128], bf16, tag="wqo")
            w_ke = wgt.tile([128, NCH, 128], bf16, tag="wke")
            w_ko = wgt.tile([128, NCH, 128], bf16, tag="wko")
            w_v = wgt.tile([128, NCH, OC], bf16, tag="wv")
            w_o = wgt.tile([128, 2, C], bf16, tag="wo")
            t_cos = wgt.tile([128, T], f32r, tag="cos")
            t_sin = wgt.tile([128, T], f32r, tag="sin")
            t_tri = wgt.tile([128, 128], bf16, tag="tri")
            t_one = wgt.tile([128, 64], f32r, tag="one")
            t_me = wgt.tile([128, 256], f32r, tag="me")
            t_mo = wgt.tile([128, 256], f32r, tag="mo")

            # DMA issue order matters: x(tb0) + qk weights first so PE can
            # start, everything else behind them.
            xr = xt.rearrange("(a p) t -> p a t", p=128)
            nc.sync.dma_start(out=Xt[:, :, 0], in_=xr[:, :, 0:512])
            nc.scalar.dma_start(out=w_qe, in_=wqe.rearrange("(a p) m -> p a m", p=128))
            nc.scalar.dma_start(out=w_qo, in_=wqo.rearrange("(a p) m -> p a m", p=128))
            nc.scalar.dma_start(out=w_ke, in_=wke.rearrange("(a p) m -> p a m", p=128))
            nc.scalar.dma_start(out=w_ko, in_=wko.rearrange("(a p) m -> p a m", p=128))
            nc.scalar.dma_start(out=t_cos[0:32, :], in_=cosd[:, :])
            nc.scalar.dma_start(out=t_sin[0:32, :], in_=sind[:, :])
            nc.scalar.dma_start(out=w_v, in_=wv.rearrange("(a p) m -> p a m", p=128))
            for r in range(1, 4):
                nc.scalar.dma_start(out=t_cos[32 * r:32 * r + 32, :], in_=t_cos[0:32, :])
                nc.scalar.dma_start(out=t_sin[32 * r:32 * r + 32, :], in_=t_sin[0:32, :])
            nc.scalar.dma_start(out=t_me, in_=pmed[:, :])
            nc.scalar.dma_start(out=t_mo, in_=pmod[:, :])
            nc.scalar.dma_start(out=t_tri, in_=trid[:, :])
            nc.scalar.dma_start(out=t_one, in_=onesd[:, :])
            for tb in range(1, NT):
                nc.sync.dma_start(out=Xt[:, :, tb],
                                  in_=xr[:, :, tb * 512:tb * 512 + 512])
            nc.scalar.dma_start(out=w_o, in_=wos.rearrange("(a p) m -> p a m", p=128))

            rQ = persist.tile([128, 2, T], f32r, tag="rq")
            rK = persist.tile([128, 2, T], f32r, tag="rk")
            Vt = persist.tile([128, NTB, HPC, D + 1], bf16, tag="v")
            OCt = persist.tile([128, 2, T], bf16, tag="oc")
            nc.vector.memset(Vt[:, :, :, D:D + 1], 1.0)

            # ---------------- projections + rope + repack --------------
            with tc.tile_pool(name="qk_ps", bufs=1, space="PSUM") as qk_ps, \
                 tc.tile_pool(name="v_ps", bufs=2, space="PSUM") as v_ps, \
                 tc.tile_pool(name="rp_ps", bufs=2, space="PSUM") as rp_ps, \
                 tc.tile_pool(name="pair", bufs=2) as pairp, \
                 tc.tile_pool(name="rope_t", bufs=2) as rope_t:
                for tb in range(NT):
                    ts = slice(tb * 512, tb * 512 + 512)
                    ps = {}
                    for nm, w in (("qe", w_qe), ("qo", w_qo),
                                  ("ke", w_ke), ("ko", w_ko)):
                        p = qk_ps.tile([128, 512], f32, tag="ps" + nm)
                        for ci in range(NCH):
                            nc.tensor.matmul(p, w[:, ci], Xt[:, ci, tb],
                                             start=(ci == 0), stop=(ci == NCH - 1))
                        ps[nm] = p
                    # rope: muls on vector (PSUM reads), add/sub on gpsimd
                    pair = {}
                    for nm in ("q", "k"):
                        e, o = ps[nm + "e"], ps[nm + "o"]
                        pe = pairp.tile([128, 512], f32r, tag="p" + nm + "e")
                        po = pairp.tile([128, 512], f32r, tag="p" + nm + "o")
                        t1 = rope_t.tile([128, 512], f32, tag="t1" + nm)
                        t2 = rope_t.tile([128, 512], f32, tag="t2" + nm)
                        nc.vector.tensor_mul(t1, e, t_cos[:, ts])
                        nc.vector.tensor_mul(t2, o, t_sin[:, ts])
                        nc.gpsimd.tensor_sub(pe, t1, t2)
                        t3 = rope_t.tile([128, 512], f32, tag="t3" + nm)
                        t4 = rope_t.tile([128, 512], f32, tag="t4" + nm)
                        nc.vector.tensor_mul(t3, o, t_cos[:, ts])
                        nc.vector.tensor_mul(t4, e, t_sin[:, ts])
                        nc.gpsimd.tensor_add(po, t3, t4)
                        pair[nm] = (pe, po)
                    # V projection: reuse the same Xt slices
                    for s in range(4):
                        pv = v_ps.tile([128, OC], f32, tag="psv",
                                       name=f"pv_{tb}_{s}")
                        for ci in range(NCH):
                            nc.tensor.matmul(
                                pv, Xt[:, ci, tb, s * 128:s * 128 + 128], w_v[:, ci],
                                start=(ci == 0), stop=(ci == NCH - 1))
                        for h in range(HPC):
                            nc.scalar.copy(
                                out=Vt[:, tb * 4 + s, h, 0:D],
                                in_=pv[:, h * D:h * D + D])
                    # repack pair -> head layout via PE permutation matmuls
                    for nm, dst in (("q", rQ), ("k", rK)):
                        pe_t, po_t = pair[nm]
                        for ob in range(2):
                            osl = slice(128 * ob, 128 * ob + 128)
                            pr = rp_ps.tile([128, 512], f32, tag="rp",
                                            name=f"rp_{nm}_{tb}_{ob}")
                            nc.tensor.matmul(pr, t_me[:, osl], pe_t,
                                             start=True, stop=False)
                            nc.tensor.matmul(pr, t_mo[:, osl], po_t,
                                             start=False, stop=True)
                            nc.scalar.copy(out=dst[:, ob, ts], in_=pr)

            # ---------------- attention, per head ----------------
            with tc.tile_pool(name="st_ps", bufs=2, space="PSUM") as st_ps, \
                 tc.tile_pool(name="ot_ps", bufs=4, space="PSUM") as ot_ps, \
                 tc.tile_pool(name="est", bufs=6) as estp, \
                 tc.tile_pool(name="nrm", bufs=3) as nrm:
                for h in range(HPC):
                    ob, rb = h // 2, 64 * (h % 2)
                    lQ = rQ[rb:rb + 64, ob, :]
                    lK = rK[rb:rb + 64, ob, :]
                    ot = [ot_ps.tile([D + 1, 512], f32, tag="ot", name=f"ot_h{h}_w{w}") for w in range(NT)]
                    est_of = {}
                    pend = []

                    def _emit_pv(c, h=h, ot=ot, est_of=est_of):
                        hs = 128 * c
                        nseg = 1 if T - hs <= 1024 else 2
                        for sg in range(nseg):
                            est, slo, w_seg = est_of[(c, sg)]
                            shi = slo + w_seg
                            for w in range(NT):
                                glo, ghi = 512 * w, 512 * w + 512
                                lo, hi = max(slo, glo), min(shi, ghi)
                                if lo >= hi:
                                    continue
                                nc.tensor.matmul(
                                    ot[w][:, lo - glo:hi - glo],
                                    Vt[:, c, h, :],
                                    est[:, lo - slo:hi - slo],
                                    start=(c == 0 and lo == glo),
                                    stop=(c == min(NTB - 1, 4 * w + 3) and hi == ghi),
                                    skip_group_check=True)

                    def _norm(w, h=h, ot=ot, ob=ob, rb=rb):
                        # normalize window w and place into OCt (bf16)
                        rl = nrm.tile([D + 1, 512], f32r, tag="rl")
                        with nc.allow_low_precision(reason="1/l in f32r feeds matmul"):
                            nc.vector.reciprocal(out=rl[D:D + 1, :], in_=ot[w][D:D + 1, :])
                        rlb = st_ps.tile([D, 512], f32, tag="st", name=f"rlb_h{h}_w{w}")
                        nc.tensor.matmul(rlb, t_one[D:D + 1, :], rl[D:D + 1, :],
                                         start=True, stop=True)
                        rlb_sb = nrm.tile([D, 512], f32, tag="rlbsb")
                        nc.vector.tensor_copy(out=rlb_sb, in_=rlb)
                        otn = nrm.tile([D, 512], bf16, tag="otn")
                        nc.vector.tensor_mul(otn, ot[w][0:D, :], rlb_sb)
                        nc.sync.dma_start(
                            out=OCt[rb:rb + D, ob, 512 * w:512 * w + 512],
                            in_=otn)

                    for c in range(NTB):
                        hs = 128 * c
                        iext = T - hs
                        nseg = 1 if iext <= 1024 else 2
                        for sg in range(nseg):
                            slo = hs + 1024 * sg              # global start
                            w_seg = min(1024, T - slo)
                            st = st_ps.tile([128, 1024], f32, tag="st")
                            # score matmuls in <=512 windows
                            off = 0
                            while off < w_seg:
                                n = min(512, w_seg - off)
                                nc.tensor.matmul(
                                    st[:, off:off + n],
                                    lK[:, hs:hs + 128],
                                    lQ[:, slo + off:slo + off + n],
                                    start=True, stop=True)
                                off += n
                            est = estp.tile([128, 1024], bf16, tag="est")
                            nc.scalar.activation(out=est[:, 0:w_seg],
                                                 in_=st[:, 0:w_seg],
                                                 func=Exp, scale=0.125)
                            if sg == 0:
                                # zero the below-diagonal part of the diag
                                # block (cheap DVE bf16 mul, off the PE)
                                nc.vector.tensor_mul(est[:, 0:128],
                                                     est[:, 0:128], t_tri)
                            est_of[(c, sg)] = (est, slo, w_seg)
                        # PV trails TWO chunks behind (software pipeline:
                        # QK stays ahead of exp so ACT never starves);
                        # normalize each window as soon as it stops.
                        pend.append(c)
                        if len(pend) > 2:
                            cc = pend.pop(0)
                            _emit_pv(cc)
                            if cc >= 3 and (cc - 3) % 4 == 0:
                                _norm((cc - 3) // 4)
                    while pend:
                        cc = pend.pop(0)
                        _emit_pv(cc)
                        if cc >= 3 and (cc - 3) % 4 == 0:
                            _norm((cc - 3) // 4)

            # ---------------- output projection ----------------
            with tc.tile_pool(name="y_ps", bufs=4, space="PSUM") as y_ps, \
                 tc.tile_pool(name="y_sb", bufs=4) as y_sb:
                for tb in range(NTB):
                    tsl = slice(tb * 128, tb * 128 + 128)
                    for ob2 in range(2):
                        py = y_ps.tile([128, 512], f32, tag="py")
                        for k in range(2):
                            nc.tensor.matmul(
                                py, OCt[:, k, tsl],
                                w_o[:, k, 512 * ob2:512 * ob2 + 512],
                                start=(k == 0), stop=(k == 1))
                        sy = y_sb.tile([128, 512], bf16, tag="sy")
                        if ob2 == 0:
                            nc.vector.tensor_copy(out=sy, in_=py)
                        else:
                            nc.scalar.copy(out=sy, in_=py)
                        (nc.sync if ob2 == 0 else nc.scalar).dma_start(
                            out=y[tsl, 512 * ob2:512 * ob2 + 512], in_=sy)
    nc.compile()
    return nc


def _prep_inputs(x, wq, wk, wv, wo, rope_cos, rope_sin):
    """Host-side sharding/pre-transposition. Core i: batch i//4, head group i%4."""
    import ml_dtypes
    f = np.float32
    bf = ml_dtypes.bfloat16
    COS = np.ascontiguousarray(rope_cos.T.astype(f))         # [32, T]
    SIN = np.ascontiguousarray(rope_sin.T.astype(f))
    tri01 = np.where(np.arange(128)[:, None] > np.arange(128)[None, :],
                     f(0.0), f(1.0)).astype(bf)
    # repack permutation for plane ob (cols ob*128..): input partition
    # 64*ob+j with j<32 -> head 2ob slab, j>=32 -> head 2ob+1 slab.
    # evens land at [0:32] / [64:96], odds at [32:64] / [96:128].
    pme = np.zeros((128, 256), f)
    pmo = np.zeros((128, 256), f)
    r = np.arange(32)
    for ob in range(2):
        pme[64 * ob + r, 128 * ob + r] = 1.0
        pme[64 * ob + 32 + r, 128 * ob + 64 + r] = 1.0
        pmo[64 * ob + r, 128 * ob + 32 + r] = 1.0
        pmo[64 * ob + 32 + r, 128 * ob + 96 + r] = 1.0
    xT = [np.ascontiguousarray(x[b].T.astype(f)).astype(bf) for b in range(B)]
    in_maps = []
    for core in range(8):
        b, g = core // 4, core % 4
        heads = [4 * g + a for a in range(HPC)]
        e_rows = np.concatenate([64 * h + 2 * np.arange(32) for h in heads])
        o_rows = e_rows + 1
        sl = slice(OC * g, OC * g + OC)
        in_maps.append({
            "xt": xT[b],
            "wqe": np.ascontiguousarray(wq[e_rows].T.astype(f)).astype(bf),
            "wqo": np.ascontiguousarray(wq[o_rows].T.astype(f)).astype(bf),
            "wke": np.ascontiguousarray(wk[e_rows].T.astype(f)).astype(bf),
            "wko": np.ascontiguousarray(wk[o_rows].T.astype(f)).astype(bf),
            "wv": np.ascontiguousarray(wv[sl].T.astype(f)).astype(bf),
            "wos": np.ascontiguousarray(wo[:, sl].T.astype(f)).astype(bf),
            "cos": COS, "sin": SIN, "tri01": tri01,
            "ones": np.ones((128, 64), np.float32),
            "pme": pme, "pmo": pmo,
        })
    return in_maps


def kernel(x, wq, wk, wv, wo, rope_cos, rope_sin, _trace=False):
    from concourse.bass_utils import run_bass_kernel_spmd
    if "nc" not in _cache:
        _cache["nc"] = _build_nc()
    nc = _cache["nc"]
    in_maps = _prep_inputs(np.asarray(x), np.asarray(wq), np.asarray(wk),
                           np.asarray(wv), np.asarray(wo),
                           np.asarray(rope_cos), np.asarray(rope_sin))
    res = run_bass_kernel_spmd(nc, in_maps, core_ids=list(range(8)),
                               trace=_trace)
    _cache["last_result"] = res
    out = np.zeros((B, T, C), np.float32)
    for core in range(8):
        out[core // 4] += np.asarray(res.results[core]["y"], np.float32)
    return out


# revision 18
# speedup vs baseline: 1.3170x; 1.0634x over previous
import sys
sys.path.insert(0, "/opt/trn_rl_repo")
import numpy as np

B, T, C, H = 2, 2048, 1024, 16
D = C // H          # 64
HPC = 4             # heads per core
OC = HPC * D        # 256 out channels per core
NEG = -1e30

_cache = {}


def _build_nc():
    import concourse.mybir as mybir
    from concourse import bacc
    from concourse.tile import TileContext

    f32 = mybir.dt.float32
    f32r = mybir.dt.float32r
    bf16 = mybir.dt.bfloat16
    Exp = mybir.ActivationFunctionType.Exp

    nc = bacc.Bacc("TRN2", target_bir_lowering=False)

    xt = nc.dram_tensor("xt", [C, T], bf16, kind="ExternalInput")
    wqe = nc.dram_tensor("wqe", [C, 128], bf16, kind="ExternalInput")
    wqo = nc.dram_tensor("wqo", [C, 128], bf16, kind="ExternalInput")
    wke = nc.dram_tensor("wke", [C, 128], bf16, kind="ExternalInput")
    wko = nc.dram_tensor("wko", [C, 128], bf16, kind="ExternalInput")
    wv = nc.dram_tensor("wv", [C, OC], bf16, kind="ExternalInput")
    wos = nc.dram_tensor("wos", [OC, C], bf16, kind="ExternalInput")
    cosd = nc.dram_tensor("cos", [32, T], f32r, kind="ExternalInput")
    sind = nc.dram_tensor("sin", [32, T], f32r, kind="ExternalInput")
    trid = nc.dram_tensor("tri01", [128, 128], bf16, kind="ExternalInput")
    onesd = nc.dram_tensor("ones", [128, 64], f32r, kind="ExternalInput")
    pmed = nc.dram_tensor("pme", [128, 256], f32r, kind="ExternalInput")
    pmod = nc.dram_tensor("pmo", [128, 256], f32r, kind="ExternalInput")
    y = nc.dram_tensor("y", [T, C], bf16, kind="ExternalOutput")

    NT = T // 512    # 4 big t-blocks
    NTB = T // 128   # 16 j-chunks / t128 blocks
    NCH = C // 128   # 8 contraction chunks

    with TileContext(nc) as tc:
        with tc.tile_pool(name="wgt", bufs=1) as wgt, \
             tc.tile_pool(name="persist", bufs=1) as persist:
            Xt = wgt.tile([128, NCH, NT, 512], bf16, tag="x")
            w_qe = wgt.tile([128, NCH, 128], bf16, tag="wqe")
            w_qo = wgt.tile([128, NCH, 128], bf16, tag="wqo")
            w_ke = wgt.tile([128, NCH, 128], bf16, tag="wke")
            w_ko = wgt.tile([128, NCH, 128], bf16, tag="wko")
            w_v = wgt.tile([128, NCH, OC], bf16, tag="wv")
            w_o = wgt.tile([128, 2, C], bf16, tag="wo")
            t_cos = wgt.tile([128, T], f32r, tag="cos")
            t_sin = wgt.tile([128, T], f32r, tag="sin")
            t_tri = wgt.tile([128, 128], bf16, tag="tri")
            t_one = wgt.tile([128, 64], f32r, tag="one")
            t_me = wgt.tile([128, 256], f32r, tag="me")
            t_mo = wgt.tile([128, 256], f32r, tag="mo")

            # DMA issue order matters: x(tb0) + qk weights first so PE can
            # start, everything else behind them.
            xr = xt.rearrange("(a p) t -> p a t", p=128)
            nc.sync.dma_start(out=Xt[:, 0:4, 0], in_=xr[:, 0:4, 0:512])
            nc.sync.dma_start(out=Xt[:, 4:8, 0], in_=xr[:, 4:8, 0:512])
            nc.scalar.dma_start(out=w_qe, in_=wqe.rearrange("(a p) m -> p a m", p=128))
            nc.scalar.dma_start(out=w_qo, in_=wqo.rearrange("(a p) m -> p a m", p=128))
            nc.scalar.dma_start(out=w_ke, in_=wke.rearrange("(a p) m -> p a m", p=128))
            nc.scalar.dma_start(out=w_ko, in_=wko.rearrange("(a p) m -> p a m", p=128))
            nc.scalar.dma_start(out=t_cos[0:32, :], in_=cosd[:, :])
            nc.scalar.dma_start(out=t_sin[0:32, :], in_=sind[:, :])
            nc.scalar.dma_start(out=w_v, in_=wv.rearrange("(a p) m -> p a m", p=128))
            nc.sync.dma_start(out=Xt[:, :, 1], in_=xr[:, :, 512:1024])
            for r in range(1, 4):
                nc.scalar.dma_start(out=t_cos[32 * r:32 * r + 32, :], in_=t_cos[0:32, :])
                nc.scalar.dma_start(out=t_sin[32 * r:32 * r + 32, :], in_=t_sin[0:32, :])
            nc.scalar.dma_start(out=t_me, in_=pmed[:, :])
            nc.scalar.dma_start(out=t_mo, in_=pmod[:, :])
            nc.scalar.dma_start(out=t_tri, in_=trid[:, :])
            nc.scalar.dma_start(out=t_one, in_=onesd[:, :])
            for tb in range(2, NT):
                nc.sync.dma_start(out=Xt[:, :, tb],
                                  in_=xr[:, :, tb * 512:tb * 512 + 512])
            nc.scalar.dma_start(out=w_o, in_=wos.rearrange("(a p) m -> p a m", p=128))

            rQ = persist.tile([128, 2, T], f32r, tag="rq")
            rK = persist.tile([128, 2, T], f32r, tag="rk")
            Vt = persist.tile([128, NTB, HPC, D + 1], bf16, tag="v")
            OCt = persist.tile([128, 2, T], bf16, tag="oc")
            nc.vector.memset(Vt[:, :, :, D:D + 1], 1.0)

            # ---------------- projections + rope + repack --------------
            with tc.tile_pool(name="qk_ps", bufs=1, space="PSUM") as qk_ps, \
                 tc.tile_pool(name="v_ps", bufs=2, space="PSUM") as v_ps, \
                 tc.tile_pool(name="rp_ps", bufs=2, space="PSUM") as rp_ps, \
                 tc.tile_pool(name="pair", bufs=2) as pairp, \
                 tc.tile_pool(name="rope_t", bufs=2) as rope_t:
                for tb in range(NT):
                    ts = slice(tb * 512, tb * 512 + 512)
                    ps = {}
                    for nm, w in (("qe", w_qe), ("qo", w_qo),
                                  ("ke", w_ke), ("ko", w_ko)):
                        p = qk_ps.tile([128, 512], f32, tag="ps" + nm)
                        for ci in range(NCH):
                            nc.tensor.matmul(p, w[:, ci], Xt[:, ci, tb],
                                             start=(ci == 0), stop=(ci == NCH - 1))
                        ps[nm] = p
                    # rope: muls on vector (PSUM reads), add/sub on gpsimd
                    pair = {}
                    for nm in ("q", "k"):
                        e, o = ps[nm + "e"], ps[nm + "o"]
                        pe = pairp.tile([128, 512], f32r, tag="p" + nm + "e")
                        po = pairp.tile([128, 512], f32r, tag="p" + nm + "o")
                        t1 = rope_t.tile([128, 512], f32, tag="t1" + nm)
                        t2 = rope_t.tile([128, 512], f32, tag="t2" + nm)
                        nc.vector.tensor_mul(t1, e, t_cos[:, ts])
                        nc.vector.tensor_mul(t2, o, t_sin[:, ts])
                        nc.gpsimd.tensor_sub(pe, t1, t2)
                        t3 = rope_t.tile([128, 512], f32, tag="t3" + nm)
                        t4 = rope_t.tile([128, 512], f32, tag="t4" + nm)
                        nc.vector.tensor_mul(t3, o, t_cos[:, ts])
                        nc.vector.tensor_mul(t4, e, t_sin[:, ts])
                        nc.gpsimd.tensor_add(po, t3, t4)
                        pair[nm] = (pe, po)
                    # V projection: reuse the same Xt slices
                    for s in range(4):
                        pv = v_ps.tile([128, OC], f32, tag="psv",
                                       name=f"pv_{tb}_{s}")
                        for ci in range(NCH):
                            nc.tensor.matmul(
                                pv, Xt[:, ci, tb, s * 128:s * 128 + 128], w_v[:, ci],
                                start=(ci == 0), stop=(ci == NCH - 1))
                        for h in range(HPC):
                            nc.scalar.copy(
                                out=Vt[:, tb * 4 + s, h, 0:D],
                                in_=pv[:, h * D:h * D + D])
                    # repack pair -> head layout via PE permutation matmuls
                    for nm, dst in (("q", rQ), ("k", rK)):
                        pe_t, po_t = pair[nm]
                        for ob in range(2):
                            osl = slice(128 * ob, 128 * ob + 128)
                            pr = rp_ps.tile([128, 512], f32, tag="rp",
                                            name=f"rp_{nm}_{tb}_{ob}")
                            nc.tensor.matmul(pr, t_me[:, osl], pe_t,
                                             start=True, stop=False)
                            nc.tensor.matmul(pr, t_mo[:, osl], po_t,
                                             start=False, stop=True)
                            nc.scalar.copy(out=dst[:, ob, ts], in_=pr)

            # ---------------- attention: flat cross-head stream ----------
            with tc.tile_pool(name="st_ps", bufs=2, space="PSUM") as st_ps, \
                 tc.tile_pool(name="ot_ps", bufs=4, space="PSUM") as ot_ps, \
                 tc.tile_pool(name="est", bufs=8) as estp, \
                 tc.tile_pool(name="nrm", bufs=3) as nrm:
                ots = {}
                est_of = {}
                pend = []    # (h, c) with QK/exp emitted, PV pending
                due = []     # (due_pop, h, w, rl) deferred normalize-B
                state = {"pop": 0}

                def _emit_pv(h, c):
                    hs = 128 * c
                    nseg = 1 if T - hs <= 1024 else 2
                    for sg in range(nseg):
                        est, slo, w_seg = est_of.pop((h, c, sg))
                        shi = slo + w_seg
                        for w in range(NT):
                            glo, ghi = 512 * w, 512 * w + 512
                            lo, hi = max(slo, glo), min(shi, ghi)
                            if lo >= hi:
                                continue
                            nc.tensor.matmul(
                                ots[h][w][:, lo - glo:hi - glo],
                                Vt[:, c, h, :],
                                est[:, lo - slo:hi - slo],
                                start=(c == 0 and lo == glo),
                                stop=(c == min(NTB - 1, 4 * w + 3) and hi == ghi),
                                skip_group_check=True)

                def _normB(h, w, rl):
                    # broadcast 1/l and scale into OCt (bf16); deferred two
                    # pops after the reciprocal so PE never waits on DVE.
                    ob, rb = h // 2, 64 * (h % 2)
                    rlb = st_ps.tile([D, 512], f32, tag="st", name=f"rlb_h{h}_w{w}")
                    nc.tensor.matmul(rlb, t_one[D:D + 1, :], rl[D:D + 1, :],
                                     start=True, stop=True)
                    rlb_sb = nrm.tile([D, 512], f32, tag="rlbsb")
                    nc.vector.tensor_copy(out=rlb_sb, in_=rlb)
                    otn = nrm.tile([D, 512], bf16, tag="otn")
                    nc.vector.tensor_mul(otn, ots[h][w][0:D, :], rlb_sb)
                    nc.sync.dma_start(
                        out=OCt[rb:rb + D, ob, 512 * w:512 * w + 512],
                        in_=otn)

                def _pop_one():
                    h, c = pend.pop(0)
                    _emit_pv(h, c)
                    state["pop"] += 1
                    while due and due[0][0] <= state["pop"]:
                        _, dh, dw, drl = due.pop(0)
                        _normB(dh, dw, drl)
                    if c >= 3 and (c - 3) % 4 == 0:
                        w = (c - 3) // 4
                        rl = nrm.tile([D + 1, 512], f32r, tag="rl")
                        with nc.allow_low_precision(reason="1/l in f32r feeds matmul"):
                            nc.vector.reciprocal(out=rl[D:D + 1, :],
                                                 in_=ots[h][w][D:D + 1, :])
                        due.append((state["pop"] + 2, h, w, rl))

                for h in range(HPC):
                    ob, rb = h // 2, 64 * (h % 2)
                    lQ = rQ[rb:rb + 64, ob, :]
                    lK = rK[rb:rb + 64, ob, :]
                    ots[h] = [ot_ps.tile([D + 1, 512], f32, tag="ot",
                                         name=f"ot_h{h}_w{w}") for w in range(NT)]
                    for c in range(NTB):
                        hs = 128 * c
                        nseg = 1 if T - hs <= 1024 else 2
                        for sg in range(nseg):
                            slo = hs + 1024 * sg              # global start
                            w_seg = min(1024, T - slo)
                            st = st_ps.tile([128, 1024], f32, tag="st")
                            off = 0
                            while off < w_seg:
                                n = min(512, w_seg - off)
                                nc.tensor.matmul(
                                    st[:, off:off + n],
                                    lK[:, hs:hs + 128],
                                    lQ[:, slo + off:slo + off + n],
                                    start=True, stop=True)
                                off += n
                            est = estp.tile([128, 1024], bf16, tag="est")
                            nc.scalar.activation(out=est[:, 0:w_seg],
                                                 in_=st[:, 0:w_seg],
                                                 func=Exp, scale=0.125)
                            if sg == 0:
                                # zero below-diagonal of the diag block
                                # (cheap DVE bf16 mul, off the PE)
                                nc.vector.tensor_mul(est[:, 0:128],
                                                     est[:, 0:128], t_tri)
                            est_of[(h, c, sg)] = (est, slo, w_seg)
                        pend.append((h, c))
                        if len(pend) > 2:
                            _pop_one()
                while pend:
                    _pop_one()
                while due:
                    _, dh, dw, drl = due.pop(0)
                    _normB(dh, dw, drl)

            # ---------------- output projection ----------------
            with tc.tile_pool(name="y_ps", bufs=6, space="PSUM") as y_ps, \
                 tc.tile_pool(name="y_sb", bufs=6) as y_sb:
                for tb in range(NTB):
                    tsl = slice(tb * 128, tb * 128 + 128)
                    for ob2 in range(2):
                        py = y_ps.tile([128, 512], f32, tag="py")
                        for k in range(2):
                            nc.tensor.matmul(
                                py, OCt[:, k, tsl],
                                w_o[:, k, 512 * ob2:512 * ob2 + 512],
                                start=(k == 0), stop=(k == 1))
                        sy = y_sb.tile([128, 512], bf16, tag="sy")
                        if ob2 == 0:
                            nc.vector.tensor_copy(out=sy, in_=py)
                        else:
                            nc.scalar.copy(out=sy, in_=py)
                        nc.sync.dma_start(
                            out=y[tsl, 512 * ob2:512 * ob2 + 512], in_=sy)
    nc.compile()
    return nc


def _prep_inputs(x, wq, wk, wv, wo, rope_cos, rope_sin):
    """Host-side sharding/pre-transposition. Core i: batch i//4, head group i%4."""
    import ml_dtypes
    f = np.float32
    bf = ml_dtypes.bfloat16
    COS = np.ascontiguousarray(rope_cos.T.astype(f))         # [32, T]
    SIN = np.ascontiguousarray(rope_sin.T.astype(f))
    tri01 = np.where(np.arange(128)[:, None] > np.arange(128)[None, :],
                     f(0.0), f(1.0)).astype(bf)
    # repack permutation for plane ob (cols ob*128..): input partition
    # 64*ob+j with j<32 -> head 2ob slab, j>=32 -> head 2ob+1 slab.
    # evens land at [0:32] / [64:96], odds at [32:64] / [96:128].
    pme = np.zeros((128, 256), f)
    pmo = np.zeros((128, 256), f)
    r = np.arange(32)
    for ob in range(2):
        pme[64 * ob + r, 128 * ob + r] = 1.0
        pme[64 * ob + 32 + r, 128 * ob + 64 + r] = 1.0
        pmo[64 * ob + r, 128 * ob + 32 + r] = 1.0
        pmo[64 * ob + 32 + r, 128 * ob + 96 + r] = 1.0
    xT = [np.ascontiguousarray(x[b].T.astype(f)).astype(bf) for b in range(B)]
    in_maps = []
    for core in range(8):
        b, g = core // 4, core % 4
        heads = [4 * g + a for a in range(HPC)]
        e_rows = np.concatenate([64 * h + 2 * np.arange(32) for h in heads])
        o_rows = e_rows + 1
        sl = slice(OC * g, OC * g + OC)
        in_maps.append({
            "xt": xT[b],
            "wqe": np.ascontiguousarray(wq[e_rows].T.astype(f)).astype(bf),
            "wqo": np.ascontiguousarray(wq[o_rows].T.astype(f)).astype(bf),
            "wke": np.ascontiguousarray(wk[e_rows].T.astype(f)).astype(bf),
            "wko": np.ascontiguousarray(wk[o_rows].T.astype(f)).astype(bf),
            "wv": np.ascontiguousarray(wv[sl].T.astype(f)).astype(bf),
            "wos": np.ascontiguousarray(wo[:, sl].T.astype(f)).astype(bf),
            "cos": COS, "sin": SIN, "tri01": tri01,
            "ones": np.ones((128, 64), np.float32),
            "pme": pme, "pmo": pmo,
        })
    return in_maps


def kernel(x, wq, wk, wv, wo, rope_cos, rope_sin, _trace=False):
    from concourse.bass_utils import run_bass_kernel_spmd
    if "nc" not in _cache:
        _cache["nc"] = _build_nc()
    nc = _cache["nc"]
    in_maps = _prep_inputs(np.asarray(x), np.asarray(wq), np.asarray(wk),
                           np.asarray(wv), np.asarray(wo),
                           np.asarray(rope_cos), np.asarray(rope_sin))
    res = run_bass_kernel_spmd(nc, in_maps, core_ids=list(range(8)),
                               trace=_trace)
    _cache["last_result"] = res
    out = np.zeros((B, T, C), np.float32)
    for core in range(8):
        out[core // 4] += np.asarray(res.results[core]["y"], np.float32)
    return out
